# revision 1
# baseline (speedup 1.0000x reference)
"""Trainium2 Bass kernel for nn_Attention_40407052320989.

Causal GQA attention block (Llama-style): QKV projection + RoPE + causal
softmax attention (8 KV heads, 32 Q heads, n_rep=4) + output projection.

Sharding: tensor-parallel over heads across 8 NeuronCores. Core c owns
KV head c and its 4 query heads: Wq/Wk/Wv column-sharded, Wo row-sharded
by the same head group. Each core computes a full [B, S, D] partial of
the output (its head group's contribution through Wo); the host sums the
8 partials (the row-parallel unshard).

On-chip layout: "feature on partitions, tokens on free dim" everywhere.
Scores are computed transposed (scoresT[t, s]) so the exp'd tile feeds
the PV matmul directly as the moving operand with no transposes. Softmax
sums over t (partition dim) ride a ones-vector matmul; the normalizer is
broadcast back over partitions with a rank-1 matmul and inverted as a
full-width DVE reciprocal.

Phase overlap: attention (phase 2) is Scalar-engine-bound (the exp
chain) while projections (phase 1) and the output projection (phase 3)
are TensorE-bound. Emission order interleaves P2(b0) with P1(b1) and
P2(b1) with P3(b0) so the scheduler overlaps them; PSUM is partitioned
4 banks for projections (two-pass: Q then KV, x re-streamed) and 4 for
attention, with phase 3 taking over projection banks afterwards.

RoPE pairing: Wq/Wk columns are permuted host-side so rotation pairs
(2i, 2i+1) land at partitions (i, 64+i) (rotate-half layout). Scores
are invariant to a consistent head-dim permutation of Wq and Wk. The
rotation is out = q * cos2 + swap_halves(q) * sin2n with cos2 = [cos;
cos] and sin2n = [-sin; sin]; the halves swap is two SBUF->SBUF DMAs.

Matmul operands are bf16 (measured: f32r runs ~2 cyc/row on HW, bf16 1
cyc/row); PSUM accumulation, softmax normalization, and the output stay
fp32. End-to-end numpy simulation of this precision mix: 4e-3 max rel.
"""

import math
import sys

sys.path.insert(0, "/opt/trn_rl_repo")

import ml_dtypes
import numpy as np

import concourse.bass as bass
import concourse.mybir as mybir
import concourse.tile as tile
from concourse import bacc
from concourse.bass_utils import run_bass_kernel_spmd

F32 = mybir.dt.float32
F32R = mybir.dt.float32r
BF16 = mybir.dt.bfloat16
AF = mybir.ActivationFunctionType
NP_BF16 = ml_dtypes.bfloat16

BSZ, SEQLEN, DIM = 2, 2048, 4096
N_HEADS, N_KV_HEADS, HEAD_DIM = 32, 8, 128
N_REP = N_HEADS // N_KV_HEADS  # q heads per core
N_CORES = 8
P = 128
NKD = DIM // P          # 32 contraction chunks for the projections
NT512 = SEQLEN // 512   # 4 blocks of 512 tokens per batch
NTC = SEQLEN // P       # 16 chunks of 128 tokens per batch
SCALE = 1.0 / math.sqrt(HEAD_DIM)

_CACHED = {}


def ts(i, n):
    return slice(i * n, (i + 1) * n)


class _Ctx:
    """Shared tiles/pools threaded through the phase emitters."""


def _build_nc():
    nc = bacc.Bacc(None, target_bir_lowering=False, debug=False)

    c = _Ctx()
    c.nc = nc
    c.xT = nc.declare_dram_parameter("xT", [BSZ, DIM, SEQLEN], BF16, isOutput=False)
    c.wq = nc.declare_dram_parameter("wq", [DIM, N_REP * HEAD_DIM], BF16, isOutput=False)
    c.wkv = nc.declare_dram_parameter("wkv", [DIM, 2 * HEAD_DIM], BF16, isOutput=False)
    c.wo = nc.declare_dram_parameter("wo", [N_REP * HEAD_DIM, DIM], BF16, isOutput=False)
    cos2 = nc.declare_dram_parameter("cos2", [P, SEQLEN], F32, isOutput=False)
    sin2n = nc.declare_dram_parameter("sin2n", [P, SEQLEN], F32, isOutput=False)
    dmask = nc.declare_dram_parameter("dmask", [P, 4, 512], BF16, isOutput=False)
    ident = nc.declare_dram_parameter("ident", [P, P], BF16, isOutput=False)
    ones_c = nc.declare_dram_parameter("ones_c", [P, 1], BF16, isOutput=False)
    ones_r = nc.declare_dram_parameter("ones_r", [1, P], F32R, isOutput=False)
    c.y = nc.declare_dram_parameter("y", [BSZ, SEQLEN, DIM], F32, isOutput=True)

    with tile.TileContext(nc) as tc, nc.allow_low_precision(
        reason="psum accumulation and normalization stay fp32 by construction"
    ):
        c.tc = tc
        with tc.tile_pool(name="const", bufs=1) as cpool:
            c.cos_sb = cpool.tile([P, SEQLEN], F32)
            c.sin_sb = cpool.tile([P, SEQLEN], F32)
            c.dm_sb = cpool.tile([P, 4, 512], BF16)
            c.id_sb = cpool.tile([P, P], BF16)
            c.onec_sb = cpool.tile([P, 1], BF16)
            c.oner_sb = cpool.tile([1, P], F32R)
            nc.sync.dma_start(c.cos_sb[:], cos2[:])
            nc.sync.dma_start(c.sin_sb[:], sin2n[:])
            nc.sync.dma_start(c.dm_sb[:], dmask[:])
            nc.sync.dma_start(c.id_sb[:], ident[:])
            nc.sync.dma_start(c.onec_sb[:], ones_c[:])
            nc.sync.dma_start(c.oner_sb[:], ones_r[:])
            _emit(c)

    nc.compile()
    return nc


def _emit(c):
    nc, tc = c.nc, c.tc
    with tc.tile_pool(name="big", bufs=1) as big, \
         tc.tile_pool(name="xs", bufs=8) as xpool, \
         tc.tile_pool(name="tp", bufs=2) as tpool, \
         tc.tile_pool(name="ep", bufs=6) as epool, \
         tc.tile_pool(name="np_", bufs=2) as npool:
        c.xpool, c.tpool, c.epool, c.npool = xpool, tpool, epool, npool

        c.wq_sb = big.tile([P, NKD, N_REP * HEAD_DIM], BF16)
        c.wkv_sb = big.tile([P, NKD, 2 * HEAD_DIM], BF16)
        c.wo_sb = big.tile([P, N_REP, DIM], BF16)
        wq_r = c.wq.rearrange("(o p) m -> p o m", p=P)
        wkv_r = c.wkv.rearrange("(o p) m -> p o m", p=P)
        for o4 in range(4):
            nc.sync.dma_start(c.wq_sb[:, ts(o4, 8), :], wq_r[:, ts(o4, 8), :])
            nc.sync.dma_start(c.wkv_sb[:, ts(o4, 8), :], wkv_r[:, ts(o4, 8), :])

        for b in range(2):
            with tc.tile_pool(name=f"qkv{b}", bufs=1) as qkv:
                c.qt = {b: qkv.tile([P, N_REP, SEQLEN], BF16, name=f"qt{b}")}
                c.kt = {b: qkv.tile([P, SEQLEN], BF16, name=f"kt{b}")}
                c.vn = {b: qkv.tile([P, NTC, HEAD_DIM], BF16, name=f"vn{b}")}
                c.ao = {b: qkv.tile([P, N_REP, SEQLEN], BF16, name=f"ao{b}")}
                with tc.tile_pool(name=f"p1_{b}", bufs=1, space="PSUM") as p1:
                    c.p1 = p1
                    for t5 in range(NT512):
                        _p1_block(c, b, t5)
                with tc.tile_pool(name=f"p2_{b}", bufs=1, space="PSUM") as p2:
                    c.p2 = p2
                    for h in range(N_REP):
                        _p2_head(c, b, h)
                if b == 0:
                    wo_r = c.wo.rearrange("(o p) n -> p o n", p=P)
                    for o4 in range(4):
                        nc.sync.dma_start(c.wo_sb[:, o4, :], wo_r[:, o4, :])
                with tc.tile_pool(name=f"p3_{b}", bufs=1, space="PSUM") as p3, \
                     tc.tile_pool(name=f"op{b}", bufs=3) as opool:
                    c.p3, c.opool = p3, opool
                    for s1 in range(NTC):
                        _p3_row(c, b, s1)


def _rope(c, out_slice, psum_in, tsl):
    """out = psum_in * cos2 + swap_halves(psum_in) * sin2n, [128, 512]."""
    nc = c.nc
    qf = c.tpool.tile([P, 512], F32, tag="rope_qf")
    rot = c.tpool.tile([P, 512], F32, tag="rope_rot")
    tmpa = c.tpool.tile([P, 512], F32, tag="rope_tmpa")
    nc.any.tensor_copy(qf[:], psum_in[:])
    nc.sync.dma_start(rot[0:64, :], qf[64:128, :])
    nc.sync.dma_start(rot[64:128, :], qf[0:64, :])
    nc.vector.tensor_mul(tmpa[:], qf[:], c.cos_sb[:, tsl])
    nc.vector.tensor_mul(rot[:], rot[:], c.sin_sb[:, tsl])
    nc.vector.tensor_add(out_slice, tmpa[:], rot[:])


def _p1_block(c, b, t5):
    """Projections for one 512-token block (single pass, 6 accumulators)."""
    nc = c.nc
    tsl = ts(t5, 512)
    pq = [c.p1.tile([P, 512], F32, tag=f"pq{h}", name=f"pq{h}")
          for h in range(N_REP)]
    pk = c.p1.tile([P, 512], F32, tag="pk")
    pv = c.p1.tile([P, 512], F32, tag="pv")
    for kd in range(NKD):
        xt = c.xpool.tile([P, 512], BF16, tag="xt")
        nc.sync.dma_start(xt[:], c.xT[b, ts(kd, P), tsl])
        st, sp = kd == 0, kd == NKD - 1
        for h in range(N_REP):
            nc.tensor.matmul(pq[h][:], c.wq_sb[:, kd, ts(h, P)], xt[:],
                             start=st, stop=sp)
        nc.tensor.matmul(pk[:], c.wkv_sb[:, kd, 0:P], xt[:], start=st, stop=sp)
        nc.tensor.matmul(pv[:], c.wkv_sb[:, kd, P:2 * P], xt[:], start=st, stop=sp)
    _rope(c, c.kt[b][:, tsl], pk, tsl)
    for h in range(N_REP):
        _rope(c, c.qt[b][:, h, tsl], pq[h], tsl)
    # V^T [d, t] -> V natural [t, d] via PE transpose
    vt_tmp = c.tpool.tile([P, 512], BF16, tag="vt_tmp")
    nc.any.tensor_copy(vt_tmp[:], pv[:])
    for j in range(4):
        pvt = c.p1.tile([P, P], BF16, tag="pvt", name="pvt")
        nc.tensor.transpose(pvt[:], vt_tmp[:, ts(j, P)], c.id_sb[:])
        nc.any.tensor_copy(c.vn[b][:, t5 * 4 + j, :], pvt[:])


def _p2_head(c, b, h):
    """Causal attention for one query head, scores transposed [t, s]."""
    nc = c.nc
    for s5 in range(NT512):
        po = c.p2.tile([P, 512], F32, tag="po", bufs=2)
        pz = c.p2.tile([1, 512], F32, tag="pzb", bufs=2, name="pz")
        ssl = ts(s5, 512)
        ntc = 4 * s5 + 4
        for tci in range(ntc):
            pscr = c.p2.tile([P, 512], F32, tag="ps", bufs=4, name="pscr")
            nc.tensor.matmul(pscr[:], c.kt[b][:, ts(tci, P)], c.qt[b][:, h, ssl],
                             start=True, stop=True)
            ex = c.epool.tile([P, 512], BF16, tag="ex")
            nc.scalar.activation(ex[:], pscr[:], AF.Exp, scale=SCALE)
            if tci >= 4 * s5:
                nc.gpsimd.tensor_mul(ex[:], ex[:], c.dm_sb[:, tci - 4 * s5, :])
            st, sp = tci == 0, tci == ntc - 1
            nc.tensor.matmul(po[:], c.vn[b][:, tci, :], ex[:], start=st, stop=sp)
            nc.tensor.matmul(pz[:], c.onec_sb[:], ex[:], start=st, stop=sp)
        # broadcast sums over partitions, then invert at full width
        zs = c.npool.tile([1, 512], F32R, tag="zs")
        nc.vector.tensor_copy(zs[:], pz[:])
        pb = c.p2.tile([P, 512], F32, tag="pzb", bufs=2, name="pb")
        nc.tensor.matmul(pb[:], c.oner_sb[:], zs[:], start=True, stop=True)
        rb = c.npool.tile([P, 512], F32, tag="rb")
        nc.vector.reciprocal(rb[:], pb[:])
        nc.vector.tensor_mul(c.ao[b][:, h, ssl], po[:], rb[:])


def _p3_row(c, b, s1):
    """Output projection for one 128-token row: two [128, 1024] groups x2."""
    nc = c.nc
    for half in range(4):
        pf = c.p3.tile([P, 2, 512], F32, tag="pf", bufs=2)
        for nq in range(2):
            n5 = half * 2 + nq
            for dh in range(N_REP):
                nc.tensor.matmul(pf[:, nq, :], c.ao[b][:, dh, ts(s1, P)],
                                 c.wo_sb[:, dh, ts(n5, 512)],
                                 start=dh == 0, stop=dh == N_REP - 1)
        ot = c.opool.tile([P, 1024], F32, tag="ot")
        nc.vector.tensor_copy(ot[:], pf[:])
        nc.sync.dma_start(c.y[b, ts(s1, P), ts(half, 1024)], ot[:])


def _prep_inputs(x, freqs_cos, freqs_sin, Wq, Wk, Wv, Wo):
    x = np.ascontiguousarray(np.asarray(x, dtype=np.float32))
    Wq = np.asarray(Wq, dtype=np.float32)
    Wk = np.asarray(Wk, dtype=np.float32)
    Wv = np.asarray(Wv, dtype=np.float32)
    Wo = np.asarray(Wo, dtype=np.float32)
    fc = np.asarray(freqs_cos, dtype=np.float32)
    fs = np.asarray(freqs_sin, dtype=np.float32)

    xT = np.ascontiguousarray(x.transpose(0, 2, 1)).astype(NP_BF16)  # [B, D, S]

    # rotate-half column permutation within each head
    perm = np.concatenate([np.arange(0, HEAD_DIM, 2), np.arange(1, HEAD_DIM, 2)])

    cos2 = np.concatenate([fc.T, fc.T], axis=0)       # [128, S]
    sin2n = np.concatenate([-fs.T, fs.T], axis=0)     # [128, S]

    # dmask[p, k, j] = 1 if j >= p + 128*k  (valid, t <= s inside diag block)
    jj = np.arange(512)[None, None, :]
    pp = np.arange(P)[:, None, None]
    kk = np.arange(4)[None, :, None]
    dmask = (jj >= pp + P * kk).astype(NP_BF16)

    ident = np.eye(P, dtype=NP_BF16)
    ones_c = np.ones((P, 1), NP_BF16)
    ones_r = np.ones((1, P), np.float32)

    in_maps = []
    for c in range(N_CORES):
        qcols = np.concatenate(
            [(4 * c + h) * HEAD_DIM + perm for h in range(N_REP)])
        kcols = c * HEAD_DIM + perm
        vcols = c * HEAD_DIM + np.arange(HEAD_DIM)
        wq_c = np.ascontiguousarray(Wq[:, qcols]).astype(NP_BF16)
        wkv_c = np.ascontiguousarray(
            np.concatenate([Wk[:, kcols], Wv[:, vcols]], axis=1)).astype(NP_BF16)
        wo_c = np.ascontiguousarray(
            Wo[c * N_REP * HEAD_DIM:(c + 1) * N_REP * HEAD_DIM, :]).astype(NP_BF16)
        in_maps.append({
            "xT": xT, "wq": wq_c, "wkv": wkv_c, "wo": wo_c,
            "cos2": cos2, "sin2n": sin2n, "dmask": dmask,
            "ident": ident, "ones_c": ones_c, "ones_r": ones_r,
        })
    return in_maps


def get_nc():
    if "nc" not in _CACHED:
        _CACHED["nc"] = _build_nc()
    return _CACHED["nc"]


def kernel(x, start_pos, freqs_cos, freqs_sin, mask, cache_k, cache_v,
           Wq, Wk, Wv, Wo, _trace=False, _tmpdir=None):
    assert int(start_pos) == 0, "kernel hardcodes start_pos == 0"
    nc = get_nc()
    in_maps = _prep_inputs(x, freqs_cos, freqs_sin, Wq, Wk, Wv, Wo)
    kwargs = {}
    if _trace:
        kwargs = {"trace": True, "tmpdir": _tmpdir}
    res = run_bass_kernel_spmd(nc, in_maps, core_ids=list(range(N_CORES)), **kwargs)
    out = res.results[0]["y"].astype(np.float64)
    for c in range(1, N_CORES):
        out += res.results[c]["y"]
    out = out.astype(np.float32)
    if _trace:
        return out, res
    return out



# revision 53
# speedup vs baseline: 1.3196x; 1.3196x over previous
"""Trainium2 Bass kernel for nn_Attention_40407052320989.

Causal GQA attention block (Llama-style): QKV projection + RoPE + causal
softmax attention (8 KV heads, 32 Q heads, n_rep=4) + output projection.

Sharding: tensor-parallel over heads across 8 NeuronCores. Core c owns
KV head c and its 4 query heads: Wq/Wk/Wv column-sharded, Wo row-sharded
by the same head group. Each core computes a full [B, S, D] partial of
the output (its head group's contribution through Wo); the host sums the
8 partials (the row-parallel unshard).

Schedule: 4 phases, generator-interleaved so the PE never idles long
enough to drop to the cold HAM clock:
  A: projections(b0)          (PE-dense)
  B: attention(b0) x outproj(b0)   (outproj groups fill the exp-gated gaps)
  C: projections(b1)          (PE-dense)
  D: attention(b1) x outproj(b1)

PE stream is near-minimal: no per-chunk softmax-sum matmuls. The softmax
denominator is accumulated on the Vector engine (exsum, f32) and
reduced+broadcast across partitions with a single all-ones matmul per
(head, s5-block); 1/z uses the fast approx DVE reciprocal. Each head's
z-reduce + normalize tail is deferred into the next head's chunk stream
so the in-order PE queue never stalls on the Vector chain. V is
projected directly into natural [t, d] layout by swapping matmul roles
(xt chunks stationary; only the bank's first matmul carries start=True
since start clears has_written for the whole bank). Diagonal score
blocks run at partial width with the causal -1e9 mask folded into PSUM
via an identity-stationary matmul, so exp() zeroes the dead region and
no post-exp mask op sits on the critical path. qt/kt/vn are per-block
tiles so phase B never waits on phase A's final rope; batch 0 keeps 8
output-projection groups in reserve to cover phase D's PSUM pool-open
barrier.

RoPE pairing: Wq/Wk columns are permuted host-side so rotation pairs
(2i, 2i+1) land at partitions (i, 64+i) (rotate-half layout). The
rotation is out = q * cos2 + swap_halves(q) * sin2n with cos2 = [cos;
cos] and sin2n = [-sin; sin]; the halves swap is two SBUF->SBUF DMAs.

Matmul operands are bf16 (f32r only for the z reduce); PSUM accumulation
and the softmax normalizer stay fp32. Output partials are written bf16
and summed fp32 on the host.
"""

import math
import sys

sys.path.insert(0, "/opt/trn_rl_repo")

import ml_dtypes
import numpy as np

import concourse.bass as bass
import concourse.mybir as mybir
import concourse.tile as tile
from concourse import bacc
from concourse.bass_utils import run_bass_kernel_spmd

F32 = mybir.dt.float32
F32R = mybir.dt.float32r
BF16 = mybir.dt.bfloat16
AF = mybir.ActivationFunctionType
NP_BF16 = ml_dtypes.bfloat16

BSZ, SEQLEN, DIM = 2, 2048, 4096
N_HEADS, N_KV_HEADS, HEAD_DIM = 32, 8, 128
N_REP = N_HEADS // N_KV_HEADS  # q heads per core
N_CORES = 8
P = 128
NKD = DIM // P          # 32 contraction chunks for the projections
NT512 = SEQLEN // 512   # 4 blocks of 512 tokens per batch
NTC = SEQLEN // P       # 16 chunks of 128 tokens per batch
SCALE = 1.0 / math.sqrt(HEAD_DIM)

_CACHED = {}


def ts(i, n):
    return slice(i * n, (i + 1) * n)


class _Ctx:
    """Shared tiles/pools threaded through the phase emitters."""


def _build_nc():
    nc = bacc.Bacc(None, target_bir_lowering=False, debug=False)

    c = _Ctx()
    c.nc = nc
    c.xT = nc.declare_dram_parameter("xT", [BSZ, DIM, SEQLEN], BF16, isOutput=False)
    c.wq = nc.declare_dram_parameter("wq", [DIM, N_REP * HEAD_DIM], BF16, isOutput=False)
    c.wkv = nc.declare_dram_parameter("wkv", [DIM, 2 * HEAD_DIM], BF16, isOutput=False)
    c.wo = nc.declare_dram_parameter("wo", [N_REP * HEAD_DIM, DIM], BF16, isOutput=False)
    cos2 = nc.declare_dram_parameter("cos2", [P, SEQLEN], BF16, isOutput=False)
    sin2n = nc.declare_dram_parameter("sin2n", [P, SEQLEN], BF16, isOutput=False)
    ident = nc.declare_dram_parameter("ident", [P, P], BF16, isOutput=False)
    trineg = nc.declare_dram_parameter("trineg", [P, P], BF16, isOutput=False)
    allones = nc.declare_dram_parameter("allones", [P, P], F32R, isOutput=False)
    c.y = nc.declare_dram_parameter("y", [BSZ, SEQLEN, DIM], BF16, isOutput=True)

    with tile.TileContext(nc) as tc, nc.allow_low_precision(
        reason="psum accumulation and normalization stay fp32 by construction"
    ):
        c.tc = tc
        with tc.tile_pool(name="const", bufs=1) as cpool:
            c.cos_sb = cpool.tile([P, SEQLEN], BF16)
            c.sin_sb = cpool.tile([P, SEQLEN], BF16)
            c.id_sb = cpool.tile([P, P], BF16)
            c.trn_sb = cpool.tile([P, P], BF16)
            c.ones_sb = cpool.tile([P, P], F32R)
            nc.sync.dma_start(c.cos_sb[:], cos2[:])
            nc.sync.dma_start(c.sin_sb[:], sin2n[:])
            nc.sync.dma_start(c.id_sb[:], ident[:])
            nc.sync.dma_start(c.trn_sb[:], trineg[:])
            nc.sync.dma_start(c.ones_sb[:], allones[:])
            _emit(c)

    nc.compile()
    return nc


def _emit(c):
    nc, tc = c.nc, c.tc
    with tc.tile_pool(name="big", bufs=1) as big, \
         tc.tile_pool(name="xs", bufs=4) as xpool, \
         tc.tile_pool(name="tp", bufs=2) as tpool, \
         tc.tile_pool(name="ep", bufs=6) as epool, \
         tc.tile_pool(name="np_", bufs=2) as npool, \
         tc.tile_pool(name="op", bufs=3) as opool:
        c.xpool, c.tpool, c.epool, c.npool, c.opool = \
            xpool, tpool, epool, npool, opool

        c.wq_sb = big.tile([P, NKD, N_REP * HEAD_DIM], BF16)
        c.wkv_sb = big.tile([P, NKD, 2 * HEAD_DIM], BF16)
        c.wo_sb = big.tile([P, N_REP, DIM], BF16)
        c.wq_r = c.wq.rearrange("(o p) m -> p o m", p=P)
        c.wkv_r = c.wkv.rearrange("(o p) m -> p o m", p=P)
        c.wo_r = c.wo.rearrange("(o p) n -> p o n", p=P)
        xTr = c.xT.rearrange("b (o p) s -> b p o s", p=P)
        c.xTr = {b: xTr[b] for b in range(2)}

        # ao for BOTH batches persists (the phase-D reserve reads ao[0]);
        # the shared PSUM p3 pool (pf, 2 banks) spans phases B..D so
        # reserve groups start without waiting the p2-pool bank barrier.
        with tc.tile_pool(name="aop", bufs=1) as aop, \
             tc.tile_pool(name="p3s", bufs=1, space="PSUM") as p3:
            c.p3 = p3
            c.ao = {b: aop.tile([P, N_REP, SEQLEN], BF16, name=f"ao{b}")
                    for b in range(2)}
            c.reserve = []
            for b in range(2):
                with tc.tile_pool(name=f"qkv{b}", bufs=1) as qkv:
                    # per-t5-block tiles: dependencies resolve per block, so
                    # phase B's s5=0 does not wait on phase A's final rope
                    c.qt = {b: [qkv.tile([P, N_REP, 512], BF16,
                                         name=f"qt{b}_{t5}")
                                for t5 in range(NT512)]}
                    c.kt = {b: [qkv.tile([P, 512], BF16, name=f"kt{b}_{t5}")
                                for t5 in range(NT512)]}
                    c.vn = {b: [qkv.tile([P, 4, HEAD_DIM], BF16,
                                         name=f"vn{b}_{t5}")
                                for t5 in range(NT512)]}
                    # Phase A/C: projections, PE-dense
                    with tc.tile_pool(name=f"p1_{b}", bufs=1, space="PSUM") as p1:
                        c.p1 = p1
                        for _ in _gen_p1(c, b):
                            pass
                    # Phase B/D: attention interleaved with output projection
                    with tc.tile_pool(name=f"p2_{b}", bufs=1, space="PSUM") as p2:
                        c.p2 = p2
                        _drive_bd(c, b)


def _scopy(nc, out, in_):
    """PSUM->SBUF copy (with dtype cast) on the Scalar engine."""
    nc.scalar.activation(out, in_, AF.Copy)


def _rope(c, out_slice, psum_in, tsl, r):
    """out = psum_in * cos2 + swap_halves(psum_in) * sin2n, [128, 512].

    The 5 ropes of a block (r = 0..4) are split across engines/queues so
    the tail latency into the next phase is halved: element-wise work
    alternates Vector/GpSimd, the halves-swap DMAs alternate Sync/Scalar.
    """
    nc = c.nc
    qf = c.tpool.tile([P, 512], BF16, tag="rope_qf", bufs=3)
    rot = c.tpool.tile([P, 512], BF16, tag="rope_rot", bufs=3)
    tmpa = c.tpool.tile([P, 512], BF16, tag="rope_tmpa", bufs=3)
    # the PSUM->SBUF copies gate the next phase's PSUM bank reuse: split
    # them across Scalar and Vector so the tail is parallel, not serial
    if r % 2 == 0:
        _scopy(nc, qf[:], psum_in[:])
    else:
        nc.vector.tensor_copy(qf[:], psum_in[:])
    dq = nc.sync if r % 2 == 0 else nc.scalar
    dq.dma_start(rot[0:64, :], qf[64:128, :])
    dq.dma_start(rot[64:128, :], qf[0:64, :])
    eng = nc.vector if r % 2 == 0 else nc.gpsimd
    eng.tensor_mul(tmpa[:], qf[:], c.cos_sb[:, tsl])
    eng.tensor_mul(rot[:], rot[:], c.sin_sb[:, tsl])
    eng.tensor_add(out_slice, tmpa[:], rot[:])


def _gen_p1(c, b):
    """Projections for batch b. Yields after each kd chunk (~3.1K PE cyc).

    Q/K: weight-stationary, xt moving 512-wide -> pq/pk [d, t] layout.
    V:   xt-chunk-stationary, wv moving 128-wide -> natural [t, d] layout
         (4 accumulators sharing one PSUM bank), no transposes needed.
    """
    nc = c.nc
    for t5 in range(NT512):
        tsl = ts(t5, 512)
        pq = [c.p1.tile([P, 512], F32, tag=f"pq{h}", name=f"pq{h}")
              for h in range(N_REP)]
        pk = c.p1.tile([P, 512], F32, tag="pk")
        pv = c.p1.tile([P, 4, P], F32, tag="pv")
        for kd4 in range(NKD // 4):
            if b == 0 and t5 == 0:
                # interleave weight loads with compute so the first matmul
                # starts ~1us in instead of waiting for the full 9MB
                nc.sync.dma_start(c.wq_sb[:, ts(kd4, 4), :], c.wq_r[:, ts(kd4, 4), :])
                nc.sync.dma_start(c.wkv_sb[:, ts(kd4, 4), :], c.wkv_r[:, ts(kd4, 4), :])
            # one batched DMA per 4 contraction chunks, on the Activation
            # hwdge queue so x loads never serialize behind weight loads
            xt4 = c.xpool.tile([P, 4, 512], BF16, tag="xt")
            nc.scalar.dma_start(xt4[:], c.xTr[b][:, ts(kd4, 4), tsl])
            for kdi in range(4):
                kd = kd4 * 4 + kdi
                xt = xt4[:, kdi, :]
                st, sp = kd == 0, kd == NKD - 1
                _p1_kd(c, b, pq, pk, pv, xt, kd, st, sp)
                yield 3100
        _p1_tail(c, b, t5, tsl, pq, pk, pv)
        yield 0


def _p1_kd(c, b, pq, pk, pv, xt, kd, st, sp):
    nc = c.nc
    for h in range(N_REP):
        nc.tensor.matmul(pq[h][:], c.wq_sb[:, kd, ts(h, P)], xt[:],
                         start=st, stop=sp)
    nc.tensor.matmul(pk[:], c.wkv_sb[:, kd, 0:P], xt[:], start=st, stop=sp)
    for j in range(4):
        # start=True clears has_written for the WHOLE bank, so only
        # the very first matmul into this bank may carry it; the
        # other quarters' first writes overwrite-on-cleared-bit.
        nc.tensor.matmul(pv[:, j, :], xt[:, ts(j, P)],
                         c.wkv_sb[:, kd, P:2 * P],
                         start=st and j == 0, stop=sp)


def _p1_tail(c, b, t5, tsl, pq, pk, pv):
    nc = c.nc
    if b == 0 and t5 == 0:
        for o4 in range(4):
            nc.sync.dma_start(c.wo_sb[:, o4, :], c.wo_r[:, o4, :])
    _scopy(nc, c.vn[b][t5][:], pv[:])
    _rope(c, c.kt[b][t5][:], pk, tsl, 0)
    for h in range(N_REP):
        _rope(c, c.qt[b][t5][:, h, :], pq[h], tsl, h + 1)


def _gen_head(c, b, h, s5, tails):
    """Attention for one (head, 512-query-block): scores (transposed
    [t, s]) -> exp -> PV, software-pipelined one chunk deep. Yields after
    each chunk emission (a filler point: PE sees PV of the previous chunk
    + scores of the current one). Diagonal chunks run at partial width.

    The z-reduce + normalize tail is NOT emitted inline: it is appended
    to `tails` and flushed a couple of chunks into the NEXT head, so the
    in-order PE queue has scores work ahead of the z matmul while the
    final exsum add drains through the Vector queue.
    """
    nc = c.nc
    ntc = 4 * s5 + 4
    po = c.p2.tile([P, 512], F32, tag="po", bufs=4, name="po")
    # bufs=4: at s5=0 three heads run interleaved, each holding its exsum
    # across the whole block (plus one for the next head starting)
    exsum = c.npool.tile([P, 512], F32R, tag="exsum", bufs=4, name="exsum")
    prev = None  # (ex, off, tc)
    for tc_ in range(ntc):
        j = tc_ - 4 * s5
        off = max(0, P * j)
        pscr = c.p2.tile([P, 512], F32, tag="ps", bufs=2, name="pscr")
        nc.tensor.matmul(pscr[:, off:512],
                         c.kt[b][tc_ // 4][:, ts(tc_ % 4, P)],
                         c.qt[b][s5][:, h, off:512],
                         start=True, stop=j < 0)
        if j >= 0:
            # causal mask folded into PSUM: += I.T @ trineg adds -1e9 above
            # the diagonal, so exp() zeroes it -- no post-exp mask op, and
            # the PV matmul depends only on the Scalar exp
            nc.tensor.matmul(pscr[:, off:off + P], c.id_sb[:], c.trn_sb[:],
                             start=False, stop=True)
        ex = c.epool.tile([P, 512], BF16, tag="ex")
        nc.scalar.activation(ex[:, off:512], pscr[:, off:512], AF.Exp, scale=SCALE)
        if prev is not None:
            _pv_step(c, b, po, exsum, prev, last=False)
        prev = (ex, off, tc_)
        if tc_ == 2 and tails:
            tails.pop(0)()
        yield 1024 if off == 0 else 2 * (512 - off)
    _pv_step(c, b, po, exsum, prev, last=True)
    tails.append(lambda: _z_tail(c, b, h, s5, po, exsum))
    yield 512


def _z_tail(c, b, h, s5, po, exsum):
    """z = sum_t exsum broadcast to all partitions via all-ones matmul,
    then 1/z and the normalize multiply into ao."""
    nc = c.nc
    pb = c.p2.tile([P, 512], F32, tag="ps", bufs=2, name="pb")
    nc.tensor.matmul(pb[:], c.ones_sb[:], exsum[:], start=True, stop=True)
    rb = c.npool.tile([P, 512], F32, tag="rb", bufs=2, name="rb")
    nc.vector.reciprocal_approx_fast(rb[:], pb[:])
    nc.vector.tensor_mul(c.ao[b][:, h, ts(s5, 512)], po[:], rb[:])


def _pv_step(c, b, po, exsum, prev, last):
    nc = c.nc
    ex, off, tc_ = prev
    nc.tensor.matmul(po[:, off:512],
                     c.vn[b][tc_ // 4][:, tc_ % 4, :], ex[:, off:512],
                     start=tc_ == 0, stop=last)
    if tc_ == 0:
        nc.vector.tensor_copy(exsum[:], ex[:])
    else:
        # z accumulation all on Vector: GpSimd ops run ~1.15us each and its
        # slow queue drain stalls the z-reduce matmul at every head end
        nc.vector.tensor_add(exsum[:, off:512], exsum[:, off:512],
                             ex[:, off:512])


def _gen_p3_group(c, b, s1, n5, drain=False):
    """Output projection for one (128-token row, 512-dim block): 4 matmuls
    + PSUM->SBUF copy; y DMA once per n5-pair (1024-wide)."""
    nc = c.nc
    pf = c.p3.tile([P, 512], F32, tag="pf", bufs=2, name="pf")
    for dh in range(N_REP):
        nc.tensor.matmul(pf[:], c.ao[b][:, dh, ts(s1, P)],
                         c.wo_sb[:, dh, ts(n5, 512)],
                         start=dh == 0, stop=dh == N_REP - 1)
    if n5 % 2 == 0:
        c.ot = c.opool.tile([P, 1024], BF16, tag="ot", name="ot")
    # mid-phase: keep Vector clear (its backlog delays the ao normalize
    # that gates the next release); only alternate engines in the drain
    if drain and (s1 + n5) % 2 == 0:
        nc.vector.tensor_copy(c.ot[:, ts(n5 % 2, 512)], pf[:])
    else:
        _scopy(nc, c.ot[:, ts(n5 % 2, 512)], pf[:])
    if n5 % 2 == 1:
        nc.sync.dma_start(c.y[b, ts(s1, P), ts(n5 // 2, 1024)], c.ot[:])


def _drive_bd(c, b):
    """Phase B/D: attention chunks with output-projection fillers.

    s5=0 runs 3 heads round-robin (nothing to fill with yet; parallel exp
    chains hide the Scalar latency). s5>=1 runs heads sequentially with
    p3 groups paced at ~1.7x the attention PE cycles.
    """
    avail = []          # ready p3 (s1, n5) groups
    p2_cyc = [0]
    p3_cyc = [0]

    def release(s5):
        for s1 in range(4 * s5, 4 * s5 + 4):
            for n5 in range(8):
                avail.append((s1, n5))

    def fill(cap=2):
        # lag the fillers (1.5x) and cap each burst so avail never drains
        # to just-released groups whose ao is still in the normalize pipe
        n = 0
        while avail and n < cap and p3_cyc[0] < 1.5 * p2_cyc[0]:
            s1, n5 = avail.pop(0)
            _gen_p3_group(c, b, s1, n5)
            p3_cyc[0] += 2048
            n += 1

    tails = []          # deferred z-reduce + normalize closures
    # block s5's groups become fillable only once all 4 of its head tails
    # are EMITTED; h3's tail flushes inside the next block's h0, so the
    # release is deferred until a few chunks into that head
    pending = [None]

    # s5 = 0: 3-way head interleave (parallel exp chains), then h3.
    # For phase D the batch-0 reserve groups cover the pool-open barrier.
    gens = [_gen_head(c, b, h, 0, tails) for h in range(3)]
    while gens:
        for g in list(gens):
            try:
                p2_cyc[0] += next(g)
            except StopIteration:
                gens.remove(g)
        if c.reserve:
            s1, n5 = c.reserve.pop(0)
            _gen_p3_group(c, 0, s1, n5)
    for cyc in _gen_head(c, b, 3, 0, tails):
        p2_cyc[0] += cyc
        if c.reserve:
            s1, n5 = c.reserve.pop(0)
            _gen_p3_group(c, 0, s1, n5)
    while c.reserve:
        s1, n5 = c.reserve.pop(0)
        _gen_p3_group(c, 0, s1, n5)
    while len(tails) > 1:
        tails.pop(0)()
    pending[0] = 0

    for s5 in range(1, NT512):
        for h in range(N_REP):
            cnt = 0
            for cyc in _gen_head(c, b, h, s5, tails):
                p2_cyc[0] += cyc
                cnt += 1
                if h == 0:
                    # the previous block's h3 tail flushes at chunk 2;
                    # only then may its p3 groups be released
                    if cnt == 4 and pending[0] is not None:
                        release(pending[0])
                        pending[0] = None
                    if cnt <= 5:
                        continue
                fill()
        while len(tails) > 1:
            tails.pop(0)()
        pending[0] = s5
    # emit a few leftover (older-block) drain groups BEFORE flushing the
    # final z-tails: the tails' pb matmuls wait the Vector chain, and the
    # in-order PE queue must have ready work ahead of them
    for _ in range(6):
        if avail:
            s1, n5 = avail.pop(0)
            _gen_p3_group(c, b, s1, n5, drain=True)
    while tails:
        tails.pop(0)()
    if pending[0] is not None:
        release(pending[0])
    # drain remaining output projection groups; for batch 0 keep 8 back as
    # the phase-D warm-start reserve
    keep = 8 if b == 0 else 0
    while len(avail) > keep:
        s1, n5 = avail.pop(0)
        _gen_p3_group(c, b, s1, n5, drain=True)
    c.reserve = avail[:]


def _prep_inputs(x, freqs_cos, freqs_sin, Wq, Wk, Wv, Wo):
    x = np.ascontiguousarray(np.asarray(x, dtype=np.float32))
    Wq = np.asarray(Wq, dtype=np.float32)
    Wk = np.asarray(Wk, dtype=np.float32)
    Wv = np.asarray(Wv, dtype=np.float32)
    Wo = np.asarray(Wo, dtype=np.float32)
    fc = np.asarray(freqs_cos, dtype=np.float32)
    fs = np.asarray(freqs_sin, dtype=np.float32)

    xT = np.ascontiguousarray(x.transpose(0, 2, 1)).astype(NP_BF16)  # [B, D, S]

    # rotate-half column permutation within each head
    perm = np.concatenate([np.arange(0, HEAD_DIM, 2), np.arange(1, HEAD_DIM, 2)])

    cos2 = np.concatenate([fc.T, fc.T], axis=0).astype(NP_BF16)    # [128, S]
    sin2n = np.concatenate([-fs.T, fs.T], axis=0).astype(NP_BF16)  # [128, S]

    # trineg[p, j] = 0 if j >= p (valid, t <= s) else -1e9 (masked pre-exp)
    jj = np.arange(P)[None, :]
    pp = np.arange(P)[:, None]
    trineg = np.where(jj >= pp, 0.0, -1e9).astype(NP_BF16)
    ident = np.eye(P, dtype=NP_BF16)
    allones = np.ones((P, P), np.float32)

    in_maps = []
    for c in range(N_CORES):
        qcols = np.concatenate(
            [(4 * c + h) * HEAD_DIM + perm for h in range(N_REP)])
        kcols = c * HEAD_DIM + perm
        vcols = c * HEAD_DIM + np.arange(HEAD_DIM)
        wq_c = np.ascontiguousarray(Wq[:, qcols]).astype(NP_BF16)
        wkv_c = np.ascontiguousarray(
            np.concatenate([Wk[:, kcols], Wv[:, vcols]], axis=1)).astype(NP_BF16)
        wo_c = np.ascontiguousarray(
            Wo[c * N_REP * HEAD_DIM:(c + 1) * N_REP * HEAD_DIM, :]).astype(NP_BF16)
        in_maps.append({
            "xT": xT, "wq": wq_c, "wkv": wkv_c, "wo": wo_c,
            "cos2": cos2, "sin2n": sin2n, "ident": ident, "trineg": trineg,
            "allones": allones,
        })
    return in_maps


def get_nc():
    if "nc" not in _CACHED:
        _CACHED["nc"] = _build_nc()
    return _CACHED["nc"]


def kernel(x, start_pos, freqs_cos, freqs_sin, mask, cache_k, cache_v,
           Wq, Wk, Wv, Wo, _trace=False, _tmpdir=None):
    assert int(start_pos) == 0, "kernel hardcodes start_pos == 0"
    nc = get_nc()
    in_maps = _prep_inputs(x, freqs_cos, freqs_sin, Wq, Wk, Wv, Wo)
    kwargs = {}
    if _trace:
        kwargs = {"trace": True, "tmpdir": _tmpdir}
    res = run_bass_kernel_spmd(nc, in_maps, core_ids=list(range(N_CORES)), **kwargs)
    out = res.results[0]["y"].astype(np.float32)
    for c in range(1, N_CORES):
        out += res.results[c]["y"].astype(np.float32)
    if _trace:
        return out, res
    return out
